# revision 14
# baseline (speedup 1.0000x reference)
"""Trainium2 Bass kernel for nn_DotProductAttention_83476984365340.

The reference applies softmax over a singleton axis (axis=2 of a [B, T, 1]
tensor), which is identically 1.0, so the output reduces exactly to
``values.sum(axis=1)``.  The kernel is therefore a pure memory-bound
reduction over T: stream `values` [B, T, D] from HBM, sum over T.
query/keys/W are mathematically irrelevant and are not transferred.

Sharding: pure data-parallel over batch — 32 batches / 8 cores = 4 per core.

Per-core pipeline (SPMD, identical NEFF on cores 0-7):
  - 33 HWDGE DMA loads of 2 MiB (last batch: 1 MiB tail tiles) — [128,
    seg, 1024] tiles, 16 KiB contiguous DRAM read per partition, 9 slots.
  - DVE tree-adds the segments down to one [128, 1024] tile (~3.3 us,
    hidden under the ~5.8 us DMA).
  - TensorE matmul with a ones[128, 1] stationary vector reduces the 128
    partitions, accumulating the 8 tiles of a batch into PSUM [1, 1024].
  - ACT copies PSUM -> SBUF, one small DMA per batch stores to out[4, 1024].

Steady state is DMA-bound at the per-core HBM limit: neuron-profile
measures ~184-190 us on silicon (median ~185 us over repeated runs) vs a
~175 us pure-stream floor at the observed 385 GB/s in-flight DMA rate;
the residual is NEFF preamble (~8 us) and the kernel-tail drain/barrier.
"""

import numpy as np

B, T, D = 32, 4096, 1024
N_CORES = 8
BPC = B // N_CORES          # batches per core
Q = 8                       # DMA tiles per batch
ROWS = T // Q               # 512 T-rows per tile
SEG = ROWS // 128           # 4 D-row segments per partition

# T-rows per DMA tile, per batch.  The last batch ends with two 256-row
# tiles: the globally-last tile gates the serial tail (final DVE add ->
# matmul -> PSUM copy -> out DMA), so a smaller final tile shortens it.
BATCH_PLANS = [[ROWS] * Q] * (BPC - 1) + [[ROWS] * (Q - 1) + [ROWS // 2, ROWS // 2]]

_cache = {}

# test-harness hooks: extra kwargs forwarded to run_bass_kernel_spmd (e.g.
# {"trace": True}); the last BassKernelResults is stashed in _LAST_RESULT.
_RUN_KWARGS = {}
_LAST_RESULT = None


def _build_nc():
    import concourse.bacc as bacc
    import concourse.mybir as mybir
    from concourse.tile import TileContext

    f32 = mybir.dt.float32
    # Bacc (not raw Bass): its compile pipeline legalizes semaphore waits
    # (max 1 wait/instruction on TRN2, spilled to EventSemaphore insts).
    nc = bacc.Bacc()
    values = nc.declare_dram_parameter("values", [BPC, T, D], f32, isOutput=False)
    out = nc.declare_dram_parameter("out", [BPC, D], f32, isOutput=True)

    with TileContext(nc) as tc:
        with (
            tc.tile_pool(name="const", bufs=1) as const_pool,
            tc.tile_pool(name="vals", bufs=9) as vals_pool,
            tc.tile_pool(name="red", bufs=6) as red_pool,
            tc.tile_pool(name="outp", bufs=4) as out_pool,
            tc.tile_pool(name="psum", bufs=4, space="PSUM") as psum_pool,
        ):
            ones = const_pool.tile([128, 1], f32)
            nc.vector.memset(ones[:], 1.0)
            for b in range(BPC):
                plan = BATCH_PLANS[b]
                psum = psum_pool.tile([1, D], f32)
                row0 = 0
                for qi, rows in enumerate(plan):
                    seg = rows // 128
                    tile = vals_pool.tile([128, seg, D], f32)
                    src = values[b, row0:row0 + rows, :].rearrange(
                        "(p n) d -> p n d", p=128
                    )
                    row0 += rows
                    nc.sync.dma_start(out=tile[:], in_=src)
                    # tree-add the seg segments down to one [128, D] tile.
                    # The final add lands in `red` (written only by DVE) so
                    # the matmul below carries a single data dependency.
                    red = red_pool.tile([128, D], f32)
                    s = seg
                    while s > 2:
                        nc.vector.tensor_add(
                            tile[:, 0:s // 2, :], tile[:, 0:s // 2, :], tile[:, s // 2:s, :]
                        )
                        s //= 2
                    nc.vector.tensor_add(red[:], tile[:, 0, :], tile[:, 1, :])
                    # partition-reduce via ones-vector matmul, accumulate the
                    # batch's tiles in PSUM (one bank pair per batch)
                    for h in range(2):
                        nc.tensor.matmul(
                            psum[:, h * 512:(h + 1) * 512],
                            ones[:],
                            red[:, h * 512:(h + 1) * 512],
                            start=(qi == 0),
                            stop=(qi == len(plan) - 1),
                        )
                ob = out_pool.tile([1, D], f32)
                nc.scalar.copy(out=ob[:], in_=psum[:])
                nc.sync.dma_start(out=out[b:b + 1, :], in_=ob[:])
    nc.finalize()
    return nc


def kernel(query=None, keys=None, values=None, W=None, **kw):
    global _LAST_RESULT
    from concourse.bass_utils import run_bass_kernel_spmd

    if "nc" not in _cache:
        _cache["nc"] = _build_nc()
    nc = _cache["nc"]

    vals = np.ascontiguousarray(np.asarray(values), dtype=np.float32)
    assert vals.shape == (B, T, D), vals.shape
    in_maps = [{"values": vals[i * BPC:(i + 1) * BPC]} for i in range(N_CORES)]
    res = run_bass_kernel_spmd(
        nc, in_maps, core_ids=list(range(N_CORES)), **_RUN_KWARGS
    )
    _LAST_RESULT = res
    return np.concatenate([r["out"] for r in res.results], axis=0)


if __name__ == "__main__":
    rng = np.random.default_rng(0)
    v = rng.standard_normal((B, T, D), dtype=np.float32)
    got = kernel(values=v)
    want = v.sum(axis=1)
    err = np.linalg.norm(got - want) / np.linalg.norm(want)
    print("rel err vs numpy:", err)


# revision 15
# speedup vs baseline: 1.2452x; 1.2452x over previous
"""Trainium2 Bass kernel for nn_DotProductAttention_83476984365340.

The reference applies softmax over a singleton axis (axis=2 of a [B, T, 1]
tensor), which is identically 1.0, so the output reduces exactly to
``values.sum(axis=1)``.  The kernel is therefore a pure memory-bound
reduction over T: stream `values` [B, T, D] from HBM, sum over T.
query/keys/W are mathematically irrelevant and are not transferred.

Sharding: pure data-parallel over batch — 32 batches / 8 cores = 4 per core.

Per-core pipeline (SPMD, identical NEFF on cores 0-7):
  - 33 HWDGE DMA loads of 2 MiB (last batch: 1 MiB tail tiles) — [128,
    seg, 1024] tiles, 16 KiB contiguous DRAM read per partition, 9 slots.
  - DVE tree-adds the segments down to one [128, 1024] tile (~3.3 us,
    hidden under the ~5.8 us DMA).
  - TensorE matmul with a ones[128, 1] stationary vector reduces the 128
    partitions, accumulating the 8 tiles of a batch into PSUM [1, 1024].
  - ACT copies PSUM -> SBUF, one small DMA per batch stores to out[4, 1024].

Steady state is DMA-bound at the per-core HBM limit: neuron-profile
measures ~184-190 us on silicon (median ~185 us over repeated runs) vs a
~175 us pure-stream floor at the observed 385 GB/s in-flight DMA rate;
the residual is NEFF preamble (~8 us) and the kernel-tail drain/barrier.
"""

import numpy as np

B, T, D = 32, 4096, 1024
N_CORES = 8
BPC = B // N_CORES          # batches per core
Q = 8                       # DMA tiles per batch
ROWS = T // Q               # 512 T-rows per tile
SEG = ROWS // 128           # 4 D-row segments per partition

# T-rows per DMA tile, per batch.  The last batch ends with two 256-row
# tiles: the globally-last tile gates the serial tail (final DVE add ->
# matmul -> PSUM copy -> out DMA), so a smaller final tile shortens it.
BATCH_PLANS = [[ROWS] * Q] * (BPC - 1) + [[ROWS] * (Q - 1) + [ROWS // 2, ROWS // 2]]

_cache = {}

# test-harness hooks: extra kwargs forwarded to run_bass_kernel_spmd (e.g.
# {"trace": True}); the last BassKernelResults is stashed in _LAST_RESULT.
_RUN_KWARGS = {}
_LAST_RESULT = None


def _build_nc():
    import concourse.bacc as bacc
    import concourse.mybir as mybir
    from concourse.tile import TileContext

    f32 = mybir.dt.float32
    # Bacc (not raw Bass): its compile pipeline legalizes semaphore waits
    # (max 1 wait/instruction on TRN2, spilled to EventSemaphore insts).
    nc = bacc.Bacc()
    values = nc.declare_dram_parameter("values", [BPC, T, D], f32, isOutput=False)
    out = nc.declare_dram_parameter("out", [BPC, D], f32, isOutput=True)

    with TileContext(nc) as tc:
        with (
            tc.tile_pool(name="const", bufs=1) as const_pool,
            tc.tile_pool(name="vals", bufs=9) as vals_pool,
            tc.tile_pool(name="red", bufs=6) as red_pool,
            tc.tile_pool(name="outp", bufs=4) as out_pool,
            tc.tile_pool(name="psum", bufs=4, space="PSUM") as psum_pool,
        ):
            ones = const_pool.tile([128, 1], f32)
            nc.vector.memset(ones[:], 1.0)
            for b in range(BPC):
                plan = BATCH_PLANS[b]
                psum = psum_pool.tile([1, D], f32)
                row0 = 0
                for qi, rows in enumerate(plan):
                    seg = rows // 128
                    tile = vals_pool.tile([128, seg, D], f32)
                    src = values[b, row0:row0 + rows, :].rearrange(
                        "(p n) d -> p n d", p=128
                    )
                    row0 += rows
                    nc.sync.dma_start(out=tile[:], in_=src)
                    # tree-add the seg segments down to one [128, D] tile.
                    # The final add lands in `red` (written only by DVE) so
                    # the matmul below carries a single data dependency.
                    red = red_pool.tile([128, D], f32)
                    s = seg
                    while s > 2:
                        nc.vector.tensor_add(
                            tile[:, 0:s // 2, :], tile[:, 0:s // 2, :], tile[:, s // 2:s, :]
                        )
                        s //= 2
                    nc.vector.tensor_add(red[:], tile[:, 0, :], tile[:, 1, :])
                    # partition-reduce via ones-vector matmul, accumulate the
                    # batch's tiles in PSUM (one bank pair per batch)
                    for h in range(2):
                        nc.tensor.matmul(
                            psum[:, h * 512:(h + 1) * 512],
                            ones[:],
                            red[:, h * 512:(h + 1) * 512],
                            start=(qi == 0),
                            stop=(qi == len(plan) - 1),
                        )
                ob = out_pool.tile([1, D], f32)
                nc.scalar.copy(out=ob[:], in_=psum[:])
                # out-DMA from ACT (also HWDGE, separate ring): issuing it
                # from SP would head-of-line-block the next batch's load
                # DMAs behind the ACT-copy semaphore wait (~1 us DMA gap
                # per batch boundary on clean runs).
                nc.scalar.dma_start(out=out[b:b + 1, :], in_=ob[:])
    nc.finalize()
    return nc


def kernel(query=None, keys=None, values=None, W=None, **kw):
    global _LAST_RESULT
    from concourse.bass_utils import run_bass_kernel_spmd

    if "nc" not in _cache:
        _cache["nc"] = _build_nc()
    nc = _cache["nc"]

    vals = np.ascontiguousarray(np.asarray(values), dtype=np.float32)
    assert vals.shape == (B, T, D), vals.shape
    in_maps = [{"values": vals[i * BPC:(i + 1) * BPC]} for i in range(N_CORES)]
    res = run_bass_kernel_spmd(
        nc, in_maps, core_ids=list(range(N_CORES)), **_RUN_KWARGS
    )
    _LAST_RESULT = res
    return np.concatenate([r["out"] for r in res.results], axis=0)


if __name__ == "__main__":
    rng = np.random.default_rng(0)
    v = rng.standard_normal((B, T, D), dtype=np.float32)
    got = kernel(values=v)
    want = v.sum(axis=1)
    err = np.linalg.norm(got - want) / np.linalg.norm(want)
    print("rel err vs numpy:", err)


# revision 16
# speedup vs baseline: 1.2609x; 1.0126x over previous
"""Trainium2 Bass kernel for nn_DotProductAttention_83476984365340.

The reference applies softmax over a singleton axis (axis=2 of a [B, T, 1]
tensor), which is identically 1.0, so the output reduces exactly to
``values.sum(axis=1)``.  The kernel is therefore a pure memory-bound
reduction over T: stream `values` [B, T, D] from HBM, sum over T.
query/keys/W are mathematically irrelevant and are not transferred.

Sharding: pure data-parallel over batch — 32 batches / 8 cores = 4 per core.

Per-core pipeline (SPMD, identical NEFF on cores 0-7):
  - 33 HWDGE DMA loads of 2 MiB (last batch: 1 MiB tail tiles) — [128,
    seg, 1024] tiles, 16 KiB contiguous DRAM read per partition, 9 slots.
  - DVE tree-adds the segments down to one [128, 1024] tile (~3.3 us,
    hidden under the ~5.8 us DMA).
  - TensorE matmul with a ones[128, 1] stationary vector reduces the 128
    partitions, accumulating the 8 tiles of a batch into PSUM [1, 1024].
  - ACT copies PSUM -> SBUF, one small DMA per batch stores to out[4, 1024].

Steady state is DMA-bound at the per-core HBM limit: neuron-profile
measures ~179-180 us on silicon on a quiet chip (~210-220 us when the
shared HBM stacks are externally contended), vs a ~174 us pure-stream
floor at the observed ~409 GB/s in-flight DMA rate; the residual is the
NEFF preamble (~8 us) and the Tile kernel-exit drain/barrier.
"""

import numpy as np

B, T, D = 32, 4096, 1024
N_CORES = 8
BPC = B // N_CORES          # batches per core
Q = 8                       # DMA tiles per batch
ROWS = T // Q               # 512 T-rows per tile
SEG = ROWS // 128           # 4 D-row segments per partition

# T-rows per DMA tile, per batch.  The last batch ends with two 256-row
# tiles: the globally-last tile gates the serial tail (final DVE add ->
# matmul -> PSUM copy -> out DMA), so a smaller final tile shortens it.
BATCH_PLANS = [[ROWS] * Q] * (BPC - 1) + [[ROWS] * (Q - 1) + [ROWS // 2, ROWS // 2]]

_cache = {}

# test-harness hooks: extra kwargs forwarded to run_bass_kernel_spmd (e.g.
# {"trace": True}); the last BassKernelResults is stashed in _LAST_RESULT.
_RUN_KWARGS = {}
_LAST_RESULT = None


def _build_nc():
    import concourse.bacc as bacc
    import concourse.mybir as mybir
    from concourse.tile import TileContext

    f32 = mybir.dt.float32
    # Bacc (not raw Bass): its compile pipeline legalizes semaphore waits
    # (max 1 wait/instruction on TRN2, spilled to EventSemaphore insts).
    nc = bacc.Bacc()
    values = nc.declare_dram_parameter("values", [BPC, T, D], f32, isOutput=False)
    out = nc.declare_dram_parameter("out", [BPC, D], f32, isOutput=True)

    with TileContext(nc) as tc:
        with (
            tc.tile_pool(name="const", bufs=1) as const_pool,
            tc.tile_pool(name="vals", bufs=9) as vals_pool,
            tc.tile_pool(name="red", bufs=6) as red_pool,
            tc.tile_pool(name="outp", bufs=4) as out_pool,
            tc.tile_pool(name="psum", bufs=4, space="PSUM") as psum_pool,
        ):
            ones = const_pool.tile([128, 1], f32)
            nc.vector.memset(ones[:], 1.0)
            for b in range(BPC):
                plan = BATCH_PLANS[b]
                psum = psum_pool.tile([1, D], f32)
                row0 = 0
                for qi, rows in enumerate(plan):
                    seg = rows // 128
                    tile = vals_pool.tile([128, seg, D], f32)
                    src = values[b, row0:row0 + rows, :].rearrange(
                        "(p n) d -> p n d", p=128
                    )
                    row0 += rows
                    nc.sync.dma_start(out=tile[:], in_=src)
                    # tree-add the seg segments down to one [128, D] tile.
                    # The final add lands in `red` (written only by DVE) so
                    # the matmul below carries a single data dependency.
                    red = red_pool.tile([128, D], f32)
                    s = seg
                    while s > 2:
                        nc.vector.tensor_add(
                            tile[:, 0:s // 2, :], tile[:, 0:s // 2, :], tile[:, s // 2:s, :]
                        )
                        s //= 2
                    nc.vector.tensor_add(red[:], tile[:, 0, :], tile[:, 1, :])
                    # partition-reduce via ones-vector matmul, accumulate the
                    # batch's tiles in PSUM (one bank pair per batch)
                    for h in range(2):
                        nc.tensor.matmul(
                            psum[:, h * 512:(h + 1) * 512],
                            ones[:],
                            red[:, h * 512:(h + 1) * 512],
                            start=(qi == 0),
                            stop=(qi == len(plan) - 1),
                        )
                ob = out_pool.tile([1, D], f32)
                nc.scalar.copy(out=ob[:], in_=psum[:])
                # out-DMA from ACT (also HWDGE, separate ring): issuing it
                # from SP would head-of-line-block the next batch's load
                # DMAs behind the ACT-copy semaphore wait (~1 us DMA gap
                # per batch boundary on clean runs).
                nc.scalar.dma_start(out=out[b:b + 1, :], in_=ob[:])
    nc.finalize()
    return nc


def kernel(query=None, keys=None, values=None, W=None, **kw):
    global _LAST_RESULT
    from concourse.bass_utils import run_bass_kernel_spmd

    if "nc" not in _cache:
        _cache["nc"] = _build_nc()
    nc = _cache["nc"]

    vals = np.ascontiguousarray(np.asarray(values), dtype=np.float32)
    assert vals.shape == (B, T, D), vals.shape
    in_maps = [{"values": vals[i * BPC:(i + 1) * BPC]} for i in range(N_CORES)]
    res = run_bass_kernel_spmd(
        nc, in_maps, core_ids=list(range(N_CORES)), **_RUN_KWARGS
    )
    _LAST_RESULT = res
    return np.concatenate([r["out"] for r in res.results], axis=0)


if __name__ == "__main__":
    rng = np.random.default_rng(0)
    v = rng.standard_normal((B, T, D), dtype=np.float32)
    got = kernel(values=v)
    want = v.sum(axis=1)
    err = np.linalg.norm(got - want) / np.linalg.norm(want)
    print("rel err vs numpy:", err)
